# revision 1
# baseline (speedup 1.0000x reference)
"""Trainium2 Bass kernel for DecoderAttention (b=2, n=2048, m=1024, d=1024, h=16).

Sharding: 8 cores = 2 (batch) x 4 (head groups of 4 heads).  Each core:
  - projects q/k/v for its 4 heads from x|context (pre-transposed on host),
  - runs causal flash attention in scores-transposed layout [kj, qi]
    (softmax without max subtraction -- scores are bounded; causally masked
    entries multiply to exactly 0 after exp, matching exp(-50000)),
  - computes its partial out-projection  attn_out_g @ Wo[rows_g]  [2048, 1024].
Host sums the 4 head-group partials per batch (the "all-reduce") and adds bo.

All matmuls run in bf16 with f32 PSUM accumulation (validated ~0.4% rel err).
"""

import os

# The neuron/axon jax backend must be discoverable for the PJRT execution
# path; a JAX_PLATFORMS=cpu pin (used when running the jax reference) would
# hide the trn2 devices from this process.
if os.environ.get("JAX_PLATFORMS", "").strip().lower() == "cpu":
    del os.environ["JAX_PLATFORMS"]

from contextlib import ExitStack

import ml_dtypes
import numpy as np

import concourse.bass as bass
import concourse.tile as tile
from concourse import bacc, mybir
from concourse.bass_utils import run_bass_kernel_spmd

B, N, M, D = 2, 2048, 1024, 1024
H, DH = 16, 64
NM = N + M          # 3072 keys (self + context)
GROUPS = 4          # head groups; 4 heads = 256 cols per group
GC = 256            # columns per head group
NCORES = 8
SCALE = DH ** -0.5
P = 128
KT = D // P         # 8 contraction tiles over d
QCH = 512           # query-chunk width
NQC = N // QCH      # 4 query chunks
NKJ = NM // P       # 24 key tiles
NSELF = N // P      # 16 self key tiles
FP32 = mybir.dt.float32
F32R = mybir.dt.float32r
BF16 = mybir.dt.bfloat16
BF16NP = ml_dtypes.bfloat16


def _active_kj(c):
    """Key tiles with any unmasked entry for query chunk c (512 queries)."""
    return list(range(0, 4 * c + 4)) + list(range(NSELF, NKJ))


def _build_module(biased: bool):
    nc = bacc.Bacc(
        "TRN2",
        target_bir_lowering=False,
        debug=False,
        enable_asserts=False,
        num_devices=NCORES,
    )
    xkvT_d = nc.dram_tensor("xkvT", [D, NM], BF16, kind="ExternalInput").ap()
    wq_d = nc.dram_tensor("wq", [D, GC], BF16, kind="ExternalInput").ap()
    wk_d = nc.dram_tensor("wk", [D, GC], BF16, kind="ExternalInput").ap()
    wv_d = nc.dram_tensor("wv", [D, GC], BF16, kind="ExternalInput").ap()
    wo_d = nc.dram_tensor("wo", [GC, D], BF16, kind="ExternalInput").ap()
    msk_d = nc.dram_tensor("msk", [4 * P, QCH], BF16, kind="ExternalInput").ap()
    if biased:
        bq_d = nc.dram_tensor("bq", [1, GC], BF16, kind="ExternalInput").ap()
        bk_d = nc.dram_tensor("bk", [1, GC], BF16, kind="ExternalInput").ap()
        bv_d = nc.dram_tensor("bv", [1, GC], BF16, kind="ExternalInput").ap()
    out_d = nc.dram_tensor("out", [N, D], FP32, kind="ExternalOutput").ap()

    with tile.TileContext(nc) as tc, ExitStack() as ctx:
        const = ctx.enter_context(tc.tile_pool(name="const", bufs=1))
        pexp = ctx.enter_context(tc.tile_pool(name="pexp", bufs=6))
        bcp = ctx.enter_context(tc.tile_pool(name="bcp", bufs=3))
        # PSUM budget: 8 banks = proj(1) + bc(1) + scores(2x2) + av(2)
        ps_main = ctx.enter_context(tc.tile_pool(name="ps_main", bufs=1, space="PSUM"))
        ps_s = ctx.enter_context(tc.tile_pool(name="ps_s", bufs=2, space="PSUM"))
        ps_av = ctx.enter_context(tc.tile_pool(name="ps_av", bufs=2, space="PSUM"))

        # ---- persistent SBUF tensors (column-concatenated k-tiles) ----
        xk = const.tile([P, KT * NM], BF16)          # xkvT: 8 tiles of [128, 3072]
        wqs = const.tile([P, KT * GC], BF16)
        wks = const.tile([P, KT * GC], BF16)
        wvs = const.tile([P, KT * GC], BF16)
        wos = const.tile([P, 2 * D], BF16)           # Wo rows: 2 tiles of [128, 1024]
        mks = const.tile([P, 4 * QCH], BF16)         # 4 diagonal mask tiles
        qT = const.tile([P, 2 * N], BF16)            # [head-pair cols, qi]
        kT = const.tile([P, 2 * NM], BF16)           # [head-pair cols, kj]
        vv = const.tile([P, NKJ * 4 * 65], BF16)     # per kj tile: 4x [v(64)|1]
        aT = const.tile([P, 2 * N], BF16)            # attn_out^T, 2 k-tiles
        ones_l = const.tile([1, 64], FP32)
        if biased:
            bq_s = const.tile([1, GC], BF16)
            bk_s = const.tile([1, GC], BF16)
            bv_s = const.tile([1, GC], BF16)
            ones_row = const.tile([1, QCH], BF16)
            ones_col = const.tile([1, P], BF16)

        # ---- input DMAs ----
        for kt in range(KT):
            nc.sync.dma_start(
                xk[:, kt * NM:(kt + 1) * NM], xkvT_d[kt * P:(kt + 1) * P, :]
            )
            nc.sync.dma_start(
                wqs[:, kt * GC:(kt + 1) * GC], wq_d[kt * P:(kt + 1) * P, :]
            )
            nc.sync.dma_start(
                wks[:, kt * GC:(kt + 1) * GC], wk_d[kt * P:(kt + 1) * P, :]
            )
            nc.sync.dma_start(
                wvs[:, kt * GC:(kt + 1) * GC], wv_d[kt * P:(kt + 1) * P, :]
            )
        for kt in range(2):
            nc.sync.dma_start(wos[:, kt * D:(kt + 1) * D], wo_d[kt * P:(kt + 1) * P, :])
        nc.sync.dma_start(
            mks.rearrange("p (t q) -> p t q", t=4),
            msk_d.rearrange("(t p) q -> p t q", p=P),
        )
        nc.vector.memset(ones_l[:], 1.0)
        if biased:
            nc.sync.dma_start(bq_s[:], bq_d[:])
            nc.sync.dma_start(bk_s[:], bk_d[:])
            nc.sync.dma_start(bv_s[:], bv_d[:])
            nc.vector.memset(ones_row[:], 1.0)
            nc.vector.memset(ones_col[:], 1.0)
        # ones columns interleaved into vv: col (t*260 + h*65 + 64)
        nc.gpsimd.memset(
            vv.rearrange("p (t h x) -> p t h x", t=NKJ, h=4)[:, :, :, 64:65], 1.0
        )

        # ---- emission helpers ----
        def emit_qT_group(mt, c):
            psq = ps_main.tile([P, QCH], FP32, tag="proj", name="psq")
            for kt in range(KT):
                nc.tensor.matmul(
                    psq[:],
                    lhsT=wqs[:, kt * GC + mt * P: kt * GC + (mt + 1) * P],
                    rhs=xk[:, kt * NM + c * QCH: kt * NM + (c + 1) * QCH],
                    start=(kt == 0),
                    stop=(kt == KT - 1) and not biased,
                )
            if biased:
                nc.tensor.matmul(
                    psq[:], lhsT=bq_s[:, mt * P:(mt + 1) * P], rhs=ones_row[:],
                    start=False, stop=True,
                )
            nc.vector.tensor_copy(
                qT[:, mt * N + c * QCH: mt * N + (c + 1) * QCH], psq[:]
            )

        def emit_kT_group(mt, c2):
            psk = ps_main.tile([P, QCH], FP32, tag="proj", name="psk")
            for kt in range(KT):
                nc.tensor.matmul(
                    psk[:],
                    lhsT=wks[:, kt * GC + mt * P: kt * GC + (mt + 1) * P],
                    rhs=xk[:, kt * NM + c2 * QCH: kt * NM + (c2 + 1) * QCH],
                    start=(kt == 0),
                    stop=(kt == KT - 1) and not biased,
                )
            if biased:
                nc.tensor.matmul(
                    psk[:], lhsT=bk_s[:, mt * P:(mt + 1) * P], rhs=ones_row[:],
                    start=False, stop=True,
                )
            nc.vector.tensor_copy(
                kT[:, mt * NM + c2 * QCH: mt * NM + (c2 + 1) * QCH], psk[:]
            )

        def emit_v_group(t):
            psv = ps_main.tile([P, GC], FP32, tag="proj", name="psv")
            for kt in range(KT):
                nc.tensor.matmul(
                    psv[:],
                    lhsT=xk[:, kt * NM + t * P: kt * NM + (t + 1) * P],
                    rhs=wvs[:, kt * GC:(kt + 1) * GC],
                    start=(kt == 0),
                    stop=(kt == KT - 1) and not biased,
                )
            if biased:
                nc.tensor.matmul(
                    psv[:], lhsT=ones_col[:], rhs=bv_s[:], start=False, stop=True,
                )
            nc.vector.tensor_copy(
                vv[:, t * 260:(t + 1) * 260].rearrange("p (h x) -> p h x", h=4)[
                    :, :, 0:64
                ],
                psv.rearrange("p (h x) -> p h x", h=4),
            )

        def emit_attention_chunk(c):
            kjs = _active_kj(c)
            last = len(kjs) - 1
            for pair in range(2):
                ps_acc = [None, None]
                pending = None  # (p_tile, i) exp'd tiles not yet fed to AV

                def do_av(pt, i):
                    t = kjs[i]
                    for hh in range(2):
                        h = pair * 2 + hh
                        nc.tensor.matmul(
                            ps_acc[hh][:],
                            lhsT=vv[:, t * 260 + h * 65: t * 260 + (h + 1) * 65],
                            rhs=pt[:, hh * QCH:(hh + 1) * QCH],
                            start=(i == 0),
                            stop=(i == last),
                        )

                for i, t in enumerate(kjs):
                    # both heads' scores into one 2-bank psum tile
                    pss = ps_s.tile([P, 2 * QCH], FP32, tag="s", name="pss")
                    for hh in range(2):
                        lo, hi = hh * 64, hh * 64 + 64
                        nc.tensor.matmul(
                            pss[:, hh * QCH:(hh + 1) * QCH],
                            lhsT=kT[lo:hi, pair * NM + t * P: pair * NM + (t + 1) * P],
                            rhs=qT[lo:hi, pair * N + c * QCH: pair * N + (c + 1) * QCH],
                            start=True,
                            stop=True,
                        )
                    pt = pexp.tile([P, 2 * QCH], BF16, tag="p", name="pt")
                    nc.scalar.activation(
                        pt[:], pss[:], mybir.ActivationFunctionType.Exp
                    )
                    if 4 * c <= t < 4 * c + 4:  # diagonal tile: causal mask
                        dt = t - 4 * c
                        for hh in range(2):
                            nc.vector.tensor_mul(
                                pt[:, hh * QCH:(hh + 1) * QCH],
                                pt[:, hh * QCH:(hh + 1) * QCH],
                                mks[:, dt * QCH:(dt + 1) * QCH],
                            )
                    if i == 0:
                        ps_acc[0] = ps_av.tile([65, QCH], FP32, tag="av", name="av0")
                        ps_acc[1] = ps_av.tile([65, QCH], FP32, tag="av", name="av1")
                    if pending is not None:
                        do_av(*pending)
                    pending = (pt, i)
                do_av(*pending)

                # normalize: evict fast to free the AV psum slot, then
                # reciprocal/broadcast/multiply off the critical path on
                # lightly-loaded engines (DVE recip, ACT copy, GpSimd mul).
                for hh in range(2):
                    h = pair * 2 + hh
                    acc = ps_acc[hh]
                    unrm = bcp.tile([64, QCH], BF16, tag="unrm", name="unrm")
                    nc.vector.tensor_copy(unrm[:], acc[0:64, :])
                    den = bcp.tile([1, QCH], FP32, tag="den", name="den")
                    nc.vector.tensor_copy(den[:], acc[64:65, :])
                    rec = bcp.tile([1, QCH], FP32, tag="rec", name="rec")
                    nc.vector.reciprocal(rec[:], den[:])
                    psb = ps_main.tile([64, QCH], FP32, tag="bc", name="psb")
                    nc.tensor.matmul(
                        psb[:], lhsT=ones_l[:], rhs=rec[:], start=True, stop=True,
                    )
                    bcs = bcp.tile([64, QCH], FP32, tag="bcs", name="bcs")
                    nc.vector.tensor_copy(bcs[:], psb[:])
                    kt2 = h // 2
                    lo = (h % 2) * 64
                    nc.vector.tensor_mul(
                        aT[lo:lo + 64, kt2 * N + c * QCH: kt2 * N + (c + 1) * QCH],
                        unrm[:],
                        bcs[:],
                    )

        def emit_outproj_chunk(c):
            for it in range(4 * c, 4 * c + 4):
                for nh in range(2):
                    pso = ps_main.tile([P, QCH], FP32, tag="proj", name="pso")
                    for kt in range(2):
                        nc.tensor.matmul(
                            pso[:],
                            lhsT=aT[:, kt * N + it * P: kt * N + (it + 1) * P],
                            rhs=wos[:, kt * D + nh * QCH: kt * D + (nh + 1) * QCH],
                            start=(kt == 0),
                            stop=(kt == 1),
                        )
                    osb = pexp.tile([P, QCH], FP32, tag="osb", bufs=3, name="osb")
                    nc.vector.tensor_copy(osb[:], pso[:])
                    nc.sync.dma_start(
                        out_d[it * P:(it + 1) * P, nh * QCH:(nh + 1) * QCH], osb[:]
                    )

        # ---- interleaved emission: start attention as soon as its first
        # tiles exist; remaining projections become TensorE filler under the
        # ACT-bound attention rounds; out-projection trails each chunk ----
        for mt in range(2):
            emit_qT_group(mt, 0)
        for mt in range(2):
            emit_kT_group(mt, 0)
        for t in range(0, 4):
            emit_v_group(t)
        for c2 in (4, 5):
            for mt in range(2):
                emit_kT_group(mt, c2)
        for t in range(NSELF, NKJ):
            emit_v_group(t)
        for c in range(NQC):
            emit_attention_chunk(c)
            if c < NQC - 1:
                for mt in range(2):
                    emit_qT_group(mt, c + 1)
                for mt in range(2):
                    emit_kT_group(mt, c + 1)
                for t in range(4 * (c + 1), 4 * (c + 1) + 4):
                    emit_v_group(t)
            emit_outproj_chunk(c)

    nc.compile()
    return nc


_CACHE: dict = {}


def _module(biased: bool):
    if biased not in _CACHE:
        _CACHE[biased] = _build_module(biased)
    return _CACHE[biased]


def _mask_tiles():
    t = np.arange(4)[:, None, None]
    p = np.arange(P)[None, :, None]
    q = np.arange(QCH)[None, None, :]
    return (p + P * t <= q).astype(BF16NP).reshape(4 * P, QCH)


def kernel(x, context, Wq, bq, Wkv, bkv, Wo, bo, mask, context_mask):
    assert bool(np.all(mask)) and bool(np.all(context_mask)), (
        "only all-true padding masks are supported"
    )
    x = np.asarray(x, np.float32)
    context = np.asarray(context, np.float32)
    Wq, bq = np.asarray(Wq, np.float32), np.asarray(bq, np.float32)
    Wkv, bkv = np.asarray(Wkv, np.float32), np.asarray(bkv, np.float32)
    Wo, bo = np.asarray(Wo, np.float32), np.asarray(bo, np.float32)

    biased = bool(np.any(bq) or np.any(bkv))
    nc = _module(biased)

    msk = _mask_tiles()
    xkvT = [
        np.ascontiguousarray(
            np.concatenate([x[b], context[b]], axis=0).T.astype(BF16NP)
        )
        for b in range(B)
    ]
    in_maps = []
    for core in range(NCORES):
        b, g = divmod(core, GROUPS)
        cols = slice(g * GC, (g + 1) * GC)
        im = {
            "xkvT": xkvT[b],
            "wq": (Wq[:, cols] * SCALE).astype(BF16NP),
            "wk": Wkv[:, cols].astype(BF16NP),
            "wv": Wkv[:, D + g * GC: D + (g + 1) * GC].astype(BF16NP),
            "wo": np.ascontiguousarray(Wo[cols, :]).astype(BF16NP),
            "msk": msk,
        }
        if biased:
            im["bq"] = (bq[cols] * SCALE).astype(BF16NP).reshape(1, GC)
            im["bk"] = bkv[cols].astype(BF16NP).reshape(1, GC)
            im["bv"] = bkv[D + g * GC: D + (g + 1) * GC].astype(BF16NP).reshape(1, GC)
        in_maps.append(im)

    try:
        res = run_bass_kernel_spmd(nc, in_maps, core_ids=list(range(NCORES)))
    except ModuleNotFoundError:
        # BASS_TRACE set but the NTFF profiling hook isn't available in this
        # environment -- rerun with tracing hard-disabled.
        os.environ["BASS_NEVER_TRACE"] = "1"
        res = run_bass_kernel_spmd(nc, in_maps, core_ids=list(range(NCORES)))
    kernel.last_results = res
    out = np.zeros((B, N, D), np.float32)
    for core in range(NCORES):
        b = core // GROUPS
        out[b] += res.results[core]["out"]
    out += bo
    return out



# revision 4
# speedup vs baseline: 1.2067x; 1.2067x over previous
"""Trainium2 Bass kernel for DecoderAttention (b=2, n=2048, m=1024, d=1024, h=16).

Sharding: 8 cores = 2 (batch) x 4 (head groups of 4 heads).  Each core:
  - projects q/k/v for its 4 heads from x|context (pre-transposed on host),
  - runs causal flash attention in scores-transposed layout [kj, qi]
    (softmax without max subtraction -- scores are bounded; causally masked
    entries multiply to exactly 0 after exp, matching exp(-50000)),
  - computes its partial out-projection  attn_out_g @ Wo[rows_g]  [2048, 1024].
Host sums the 4 head-group partials per batch (the "all-reduce") and adds bo.

All matmuls run in bf16 with f32 PSUM accumulation (validated ~0.4% rel err).
"""

import os

# The neuron/axon jax backend must be discoverable for the PJRT execution
# path; a JAX_PLATFORMS=cpu pin (used when running the jax reference) would
# hide the trn2 devices from this process.
if os.environ.get("JAX_PLATFORMS", "").strip().lower() == "cpu":
    del os.environ["JAX_PLATFORMS"]

from contextlib import ExitStack

import ml_dtypes
import numpy as np

import concourse.bass as bass
import concourse.tile as tile
from concourse import bacc, mybir
from concourse.bass_utils import run_bass_kernel_spmd

B, N, M, D = 2, 2048, 1024, 1024
H, DH = 16, 64
NM = N + M          # 3072 keys (self + context)
GROUPS = 4          # head groups; 4 heads = 256 cols per group
GC = 256            # columns per head group
NCORES = 8
SCALE = DH ** -0.5
P = 128
KT = D // P         # 8 contraction tiles over d
QCH = 512           # query-chunk width
NQC = N // QCH      # 4 query chunks
NKJ = NM // P       # 24 key tiles
NSELF = N // P      # 16 self key tiles
FP32 = mybir.dt.float32
F32R = mybir.dt.float32r
BF16 = mybir.dt.bfloat16
BF16NP = ml_dtypes.bfloat16


def _active_kj(c):
    """Key tiles with any unmasked entry for query chunk c (512 queries)."""
    return list(range(0, 4 * c + 4)) + list(range(NSELF, NKJ))


def _build_module(biased: bool):
    nc = bacc.Bacc(
        "TRN2",
        target_bir_lowering=False,
        debug=False,
        enable_asserts=False,
        num_devices=NCORES,
    )
    xkvT_d = nc.dram_tensor("xkvT", [D, NM], BF16, kind="ExternalInput").ap()
    wq_d = nc.dram_tensor("wq", [D, GC], BF16, kind="ExternalInput").ap()
    wk_d = nc.dram_tensor("wk", [D, GC], BF16, kind="ExternalInput").ap()
    wv_d = nc.dram_tensor("wv", [D, GC], BF16, kind="ExternalInput").ap()
    wo_d = nc.dram_tensor("wo", [GC, D], BF16, kind="ExternalInput").ap()
    msk_d = nc.dram_tensor("msk", [4 * P, QCH], BF16, kind="ExternalInput").ap()
    if biased:
        bq_d = nc.dram_tensor("bq", [1, GC], BF16, kind="ExternalInput").ap()
        bk_d = nc.dram_tensor("bk", [1, GC], BF16, kind="ExternalInput").ap()
        bv_d = nc.dram_tensor("bv", [1, GC], BF16, kind="ExternalInput").ap()
    out_d = nc.dram_tensor("out", [N, D], FP32, kind="ExternalOutput").ap()

    with tile.TileContext(nc) as tc, ExitStack() as ctx:
        const = ctx.enter_context(tc.tile_pool(name="const", bufs=1))
        pexp = ctx.enter_context(tc.tile_pool(name="pexp", bufs=6))
        bcp = ctx.enter_context(tc.tile_pool(name="bcp", bufs=3))
        # PSUM budget: 8 banks = proj(1) + bc(1) + scores(2x2) + av(2)
        ps_main = ctx.enter_context(tc.tile_pool(name="ps_main", bufs=1, space="PSUM"))
        ps_s = ctx.enter_context(tc.tile_pool(name="ps_s", bufs=2, space="PSUM"))
        ps_av = ctx.enter_context(tc.tile_pool(name="ps_av", bufs=2, space="PSUM"))

        # ---- persistent SBUF tensors (column-concatenated k-tiles) ----
        xk = const.tile([P, KT * NM], BF16)          # xkvT: 8 tiles of [128, 3072]
        wqs = const.tile([P, KT * GC], BF16)
        wks = const.tile([P, KT * GC], BF16)
        wvs = const.tile([P, KT * GC], BF16)
        wos = const.tile([P, 2 * D], BF16)           # Wo rows: 2 tiles of [128, 1024]
        mks = const.tile([P, 4 * QCH], BF16)         # 4 diagonal mask tiles
        qT = const.tile([P, 2 * N], BF16)            # [head-pair cols, qi]
        kT = const.tile([P, 2 * NM], BF16)           # [head-pair cols, kj]
        vv = const.tile([P, NKJ * 4 * 65], BF16)     # per kj tile: 4x [v(64)|1]
        aT = const.tile([P, 2 * N], BF16)            # attn_out^T, 2 k-tiles
        ones_l = const.tile([1, 64], FP32)
        if biased:
            bq_s = const.tile([1, GC], BF16)
            bk_s = const.tile([1, GC], BF16)
            bv_s = const.tile([1, GC], BF16)
            ones_row = const.tile([1, QCH], BF16)
            ones_col = const.tile([1, P], BF16)

        # ---- input DMAs ----
        # One batched DMA per tensor/column-chunk, ordered so the first
        # projections (weights, then x columns for query-chunk 0, then the
        # context columns) unblock compute within a few us instead of after
        # the whole ~9 MB input load.
        nc.sync.dma_start(
            wqs.rearrange("p (kt g) -> p kt g", kt=KT),
            wq_d.rearrange("(kt p) g -> p kt g", p=P),
        )
        nc.sync.dma_start(
            wks.rearrange("p (kt g) -> p kt g", kt=KT),
            wk_d.rearrange("(kt p) g -> p kt g", p=P),
        )
        nc.sync.dma_start(
            wvs.rearrange("p (kt g) -> p kt g", kt=KT),
            wv_d.rearrange("(kt p) g -> p kt g", p=P),
        )
        nc.sync.dma_start(
            mks.rearrange("p (t q) -> p t q", t=4),
            msk_d.rearrange("(t p) q -> p t q", p=P),
        )
        xk_v = xk.rearrange("p (kt m) -> p kt m", kt=KT)
        xkvT_v = xkvT_d.rearrange("(kt p) m -> p kt m", p=P)
        for cc in (0, 4, 5, 1, 2, 3):
            nc.sync.dma_start(
                xk_v[:, :, cc * QCH:(cc + 1) * QCH],
                xkvT_v[:, :, cc * QCH:(cc + 1) * QCH],
            )
        nc.sync.dma_start(
            wos.rearrange("p (t d) -> p t d", t=2),
            wo_d.rearrange("(t p) d -> p t d", p=P),
        )
        nc.vector.memset(ones_l[:], 1.0)
        if biased:
            nc.sync.dma_start(bq_s[:], bq_d[:])
            nc.sync.dma_start(bk_s[:], bk_d[:])
            nc.sync.dma_start(bv_s[:], bv_d[:])
            nc.vector.memset(ones_row[:], 1.0)
            nc.vector.memset(ones_col[:], 1.0)
        # ones columns interleaved into vv: col (t*260 + h*65 + 64)
        nc.gpsimd.memset(
            vv.rearrange("p (t h x) -> p t h x", t=NKJ, h=4)[:, :, :, 64:65], 1.0
        )

        # ---- emission helpers ----
        def emit_qT_group(mt, c):
            psq = ps_main.tile([P, QCH], FP32, tag="proj", name="psq")
            for kt in range(KT):
                nc.tensor.matmul(
                    psq[:],
                    lhsT=wqs[:, kt * GC + mt * P: kt * GC + (mt + 1) * P],
                    rhs=xk[:, kt * NM + c * QCH: kt * NM + (c + 1) * QCH],
                    start=(kt == 0),
                    stop=(kt == KT - 1) and not biased,
                )
            if biased:
                nc.tensor.matmul(
                    psq[:], lhsT=bq_s[:, mt * P:(mt + 1) * P], rhs=ones_row[:],
                    start=False, stop=True,
                )
            nc.vector.tensor_copy(
                qT[:, mt * N + c * QCH: mt * N + (c + 1) * QCH], psq[:]
            )

        def emit_kT_group(mt, c2):
            psk = ps_main.tile([P, QCH], FP32, tag="proj", name="psk")
            for kt in range(KT):
                nc.tensor.matmul(
                    psk[:],
                    lhsT=wks[:, kt * GC + mt * P: kt * GC + (mt + 1) * P],
                    rhs=xk[:, kt * NM + c2 * QCH: kt * NM + (c2 + 1) * QCH],
                    start=(kt == 0),
                    stop=(kt == KT - 1) and not biased,
                )
            if biased:
                nc.tensor.matmul(
                    psk[:], lhsT=bk_s[:, mt * P:(mt + 1) * P], rhs=ones_row[:],
                    start=False, stop=True,
                )
            nc.vector.tensor_copy(
                kT[:, mt * NM + c2 * QCH: mt * NM + (c2 + 1) * QCH], psk[:]
            )

        def emit_v_group(t):
            psv = ps_main.tile([P, GC], FP32, tag="proj", name="psv")
            for kt in range(KT):
                nc.tensor.matmul(
                    psv[:],
                    lhsT=xk[:, kt * NM + t * P: kt * NM + (t + 1) * P],
                    rhs=wvs[:, kt * GC:(kt + 1) * GC],
                    start=(kt == 0),
                    stop=(kt == KT - 1) and not biased,
                )
            if biased:
                nc.tensor.matmul(
                    psv[:], lhsT=ones_col[:], rhs=bv_s[:], start=False, stop=True,
                )
            nc.vector.tensor_copy(
                vv[:, t * 260:(t + 1) * 260].rearrange("p (h x) -> p h x", h=4)[
                    :, :, 0:64
                ],
                psv.rearrange("p (h x) -> p h x", h=4),
            )

        def emit_attention_chunk(c):
            kjs = _active_kj(c)
            last = len(kjs) - 1
            for pair in range(2):
                ps_acc = [None, None]
                pending = None  # (p_tile, i) exp'd tiles not yet fed to AV

                def do_av(pt, i):
                    t = kjs[i]
                    for hh in range(2):
                        h = pair * 2 + hh
                        nc.tensor.matmul(
                            ps_acc[hh][:],
                            lhsT=vv[:, t * 260 + h * 65: t * 260 + (h + 1) * 65],
                            rhs=pt[:, hh * QCH:(hh + 1) * QCH],
                            start=(i == 0),
                            stop=(i == last),
                        )

                for i, t in enumerate(kjs):
                    # both heads' scores into one 2-bank psum tile
                    pss = ps_s.tile([P, 2 * QCH], FP32, tag="s", name="pss")
                    for hh in range(2):
                        lo, hi = hh * 64, hh * 64 + 64
                        nc.tensor.matmul(
                            pss[:, hh * QCH:(hh + 1) * QCH],
                            lhsT=kT[lo:hi, pair * NM + t * P: pair * NM + (t + 1) * P],
                            rhs=qT[lo:hi, pair * N + c * QCH: pair * N + (c + 1) * QCH],
                            start=True,
                            stop=True,
                        )
                    pt = pexp.tile([P, 2 * QCH], BF16, tag="p", name="pt")
                    nc.scalar.activation(
                        pt[:], pss[:], mybir.ActivationFunctionType.Exp
                    )
                    if 4 * c <= t < 4 * c + 4:  # diagonal tile: causal mask
                        dt = t - 4 * c
                        for hh in range(2):
                            nc.vector.tensor_mul(
                                pt[:, hh * QCH:(hh + 1) * QCH],
                                pt[:, hh * QCH:(hh + 1) * QCH],
                                mks[:, dt * QCH:(dt + 1) * QCH],
                            )
                    if i == 0:
                        ps_acc[0] = ps_av.tile([65, QCH], FP32, tag="av", name="av0")
                        ps_acc[1] = ps_av.tile([65, QCH], FP32, tag="av", name="av1")
                    if pending is not None:
                        do_av(*pending)
                    pending = (pt, i)
                do_av(*pending)

                # normalize: evict fast to free the AV psum slot; the
                # reciprocal uses the single-op approx (the precise DVE
                # reciprocal costs ~3.3us per [1,512] tile and was the
                # kernel's single largest DVE item).
                for hh in range(2):
                    h = pair * 2 + hh
                    acc = ps_acc[hh]
                    unrm = bcp.tile([64, QCH], BF16, tag="unrm", name="unrm")
                    nc.vector.tensor_copy(unrm[:], acc[0:64, :])
                    den = bcp.tile([1, QCH], FP32, tag="den", name="den")
                    nc.vector.tensor_copy(den[:], acc[64:65, :])
                    rec = bcp.tile([1, QCH], FP32, tag="rec", name="rec")
                    nc.vector.reciprocal_approx_fast(rec[:], den[:])
                    psb = ps_main.tile([64, QCH], FP32, tag="bc", name="psb")
                    nc.tensor.matmul(
                        psb[:], lhsT=ones_l[:], rhs=rec[:], start=True, stop=True,
                    )
                    bcs = bcp.tile([64, QCH], FP32, tag="bcs", name="bcs")
                    nc.vector.tensor_copy(bcs[:], psb[:])
                    kt2 = h // 2
                    lo = (h % 2) * 64
                    nc.vector.tensor_mul(
                        aT[lo:lo + 64, kt2 * N + c * QCH: kt2 * N + (c + 1) * QCH],
                        unrm[:],
                        bcs[:],
                    )

        def emit_outproj_chunk(c):
            for it in range(4 * c, 4 * c + 4):
                for nh in range(2):
                    pso = ps_main.tile([P, QCH], FP32, tag="proj", name="pso")
                    for kt in range(2):
                        nc.tensor.matmul(
                            pso[:],
                            lhsT=aT[:, kt * N + it * P: kt * N + (it + 1) * P],
                            rhs=wos[:, kt * D + nh * QCH: kt * D + (nh + 1) * QCH],
                            start=(kt == 0),
                            stop=(kt == 1),
                        )
                    osb = pexp.tile([P, QCH], FP32, tag="osb", bufs=3, name="osb")
                    # ACT evicts the out-projection psum: the scalar engine
                    # is idle during the out-projection phase (no exps in
                    # flight) while DVE is the busier engine overall.
                    nc.scalar.copy(osb[:], pso[:])
                    nc.sync.dma_start(
                        out_d[it * P:(it + 1) * P, nh * QCH:(nh + 1) * QCH], osb[:]
                    )

        # ---- interleaved emission: start attention as soon as its first
        # tiles exist; remaining projections become TensorE filler under the
        # ACT-bound attention rounds; out-projection trails each chunk ----
        for mt in range(2):
            emit_qT_group(mt, 0)
        for mt in range(2):
            emit_kT_group(mt, 0)
        for t in range(0, 4):
            emit_v_group(t)
        for c2 in (4, 5):
            for mt in range(2):
                emit_kT_group(mt, c2)
        for t in range(NSELF, NKJ):
            emit_v_group(t)
        for c in range(NQC):
            emit_attention_chunk(c)
            if c < NQC - 1:
                for mt in range(2):
                    emit_qT_group(mt, c + 1)
                for mt in range(2):
                    emit_kT_group(mt, c + 1)
                for t in range(4 * (c + 1), 4 * (c + 1) + 4):
                    emit_v_group(t)
            emit_outproj_chunk(c)

    nc.compile()
    return nc


_CACHE: dict = {}


def _module(biased: bool):
    if biased not in _CACHE:
        _CACHE[biased] = _build_module(biased)
    return _CACHE[biased]


def _mask_tiles():
    t = np.arange(4)[:, None, None]
    p = np.arange(P)[None, :, None]
    q = np.arange(QCH)[None, None, :]
    return (p + P * t <= q).astype(BF16NP).reshape(4 * P, QCH)


def kernel(x, context, Wq, bq, Wkv, bkv, Wo, bo, mask, context_mask):
    assert bool(np.all(mask)) and bool(np.all(context_mask)), (
        "only all-true padding masks are supported"
    )
    x = np.asarray(x, np.float32)
    context = np.asarray(context, np.float32)
    Wq, bq = np.asarray(Wq, np.float32), np.asarray(bq, np.float32)
    Wkv, bkv = np.asarray(Wkv, np.float32), np.asarray(bkv, np.float32)
    Wo, bo = np.asarray(Wo, np.float32), np.asarray(bo, np.float32)

    biased = bool(np.any(bq) or np.any(bkv))
    nc = _module(biased)

    msk = _mask_tiles()
    xkvT = [
        np.ascontiguousarray(
            np.concatenate([x[b], context[b]], axis=0).T.astype(BF16NP)
        )
        for b in range(B)
    ]
    in_maps = []
    for core in range(NCORES):
        b, g = divmod(core, GROUPS)
        cols = slice(g * GC, (g + 1) * GC)
        im = {
            "xkvT": xkvT[b],
            "wq": (Wq[:, cols] * SCALE).astype(BF16NP),
            "wk": Wkv[:, cols].astype(BF16NP),
            "wv": Wkv[:, D + g * GC: D + (g + 1) * GC].astype(BF16NP),
            "wo": np.ascontiguousarray(Wo[cols, :]).astype(BF16NP),
            "msk": msk,
        }
        if biased:
            im["bq"] = (bq[cols] * SCALE).astype(BF16NP).reshape(1, GC)
            im["bk"] = bkv[cols].astype(BF16NP).reshape(1, GC)
            im["bv"] = bkv[D + g * GC: D + (g + 1) * GC].astype(BF16NP).reshape(1, GC)
        in_maps.append(im)

    try:
        res = run_bass_kernel_spmd(nc, in_maps, core_ids=list(range(NCORES)))
    except ModuleNotFoundError:
        # BASS_TRACE set but the NTFF profiling hook isn't available in this
        # environment -- rerun with tracing hard-disabled.
        os.environ["BASS_NEVER_TRACE"] = "1"
        res = run_bass_kernel_spmd(nc, in_maps, core_ids=list(range(NCORES)))
    kernel.last_results = res
    out = np.zeros((B, N, D), np.float32)
    for core in range(NCORES):
        b = core // GROUPS
        out[b] += res.results[core]["out"]
    out += bo
    return out



# revision 9
# speedup vs baseline: 1.3363x; 1.1074x over previous
"""Trainium2 Bass kernel for DecoderAttention (b=2, n=2048, m=1024, d=1024, h=16).

Sharding: 8 cores = 2 (batch) x 4 (head groups of 4 heads).  Each core:
  - projects q/k/v for its 4 heads from x|context (pre-transposed on host),
  - runs causal flash attention in scores-transposed layout [kj, qi]
    (softmax without max subtraction -- scores are bounded; causally masked
    entries multiply to exactly 0 after exp, matching exp(-50000)),
  - computes its partial out-projection  attn_out_g @ Wo[rows_g]  [2048, 1024].
Host sums the 4 head-group partials per batch (the "all-reduce") and adds bo.

All matmuls run in bf16 with f32 PSUM accumulation (validated ~0.4% rel err).
"""

import os

# The neuron/axon jax backend must be discoverable for the PJRT execution
# path; a JAX_PLATFORMS=cpu pin (used when running the jax reference) would
# hide the trn2 devices from this process.
if os.environ.get("JAX_PLATFORMS", "").strip().lower() == "cpu":
    del os.environ["JAX_PLATFORMS"]

from contextlib import ExitStack

import ml_dtypes
import numpy as np

import concourse.bass as bass
import concourse.tile as tile
from concourse import bacc, mybir
from concourse.bass_utils import run_bass_kernel_spmd

B, N, M, D = 2, 2048, 1024, 1024
H, DH = 16, 64
NM = N + M          # 3072 keys (self + context)
GROUPS = 4          # head groups; 4 heads = 256 cols per group
GC = 256            # columns per head group
NCORES = 8
SCALE = DH ** -0.5
P = 128
KT = D // P         # 8 contraction tiles over d
QCH = 512           # query-chunk width
NQC = N // QCH      # 4 query chunks
NKJ = NM // P       # 24 key tiles
NSELF = N // P      # 16 self key tiles
FP32 = mybir.dt.float32
F32R = mybir.dt.float32r
BF16 = mybir.dt.bfloat16
BF16NP = ml_dtypes.bfloat16


def _active_kj(c):
    """Key tiles with any unmasked entry for query chunk c (512 queries)."""
    return list(range(0, 4 * c + 4)) + list(range(NSELF, NKJ))


def _build_module(biased: bool):
    nc = bacc.Bacc(
        "TRN2",
        target_bir_lowering=False,
        debug=False,
        enable_asserts=False,
        num_devices=NCORES,
    )
    xkvT_d = nc.dram_tensor("xkvT", [D, NM], BF16, kind="ExternalInput").ap()
    wq_d = nc.dram_tensor("wq", [D, GC], BF16, kind="ExternalInput").ap()
    wk_d = nc.dram_tensor("wk", [D, GC], BF16, kind="ExternalInput").ap()
    wv_d = nc.dram_tensor("wv", [D, GC], BF16, kind="ExternalInput").ap()
    wo_d = nc.dram_tensor("wo", [GC, D], BF16, kind="ExternalInput").ap()
    msk_d = nc.dram_tensor("msk", [4 * P, QCH], BF16, kind="ExternalInput").ap()
    if biased:
        bq_d = nc.dram_tensor("bq", [1, GC], BF16, kind="ExternalInput").ap()
        bk_d = nc.dram_tensor("bk", [1, GC], BF16, kind="ExternalInput").ap()
        bv_d = nc.dram_tensor("bv", [1, GC], BF16, kind="ExternalInput").ap()
    out_d = nc.dram_tensor("out", [N, D], FP32, kind="ExternalOutput").ap()

    with tile.TileContext(nc) as tc, ExitStack() as ctx:
        const = ctx.enter_context(tc.tile_pool(name="const", bufs=1))
        pexp = ctx.enter_context(tc.tile_pool(name="pexp", bufs=6))
        bcp = ctx.enter_context(tc.tile_pool(name="bcp", bufs=3))
        # PSUM budget: 8 banks = proj(1) + bc(1) + scores(2x2) + av(2)
        ps_main = ctx.enter_context(tc.tile_pool(name="ps_main", bufs=1, space="PSUM"))
        ps_s = ctx.enter_context(tc.tile_pool(name="ps_s", bufs=2, space="PSUM"))
        ps_av = ctx.enter_context(tc.tile_pool(name="ps_av", bufs=2, space="PSUM"))

        # ---- persistent SBUF tensors (column-concatenated k-tiles) ----
        xk = const.tile([P, KT * NM], BF16)          # xkvT: 8 tiles of [128, 3072]
        wqs = const.tile([P, KT * GC], BF16)
        wks = const.tile([P, KT * GC], BF16)
        wvs = const.tile([P, KT * GC], BF16)
        wos = const.tile([P, 2 * D], BF16)           # Wo rows: 2 tiles of [128, 1024]
        mks = const.tile([P, 4 * QCH], BF16)         # 4 diagonal mask tiles
        qT = const.tile([P, 2 * N], BF16)            # [head-pair cols, qi]
        kT = const.tile([P, 2 * NM], BF16)           # [head-pair cols, kj]
        vv = const.tile([P, NKJ * GC], BF16)         # per kj tile: 4 heads x 64
        aT = const.tile([P, 2 * N], BF16)            # attn_out^T, 2 k-tiles
        ones_bc = const.tile([P, 64], BF16)          # all-ones: den reduce+broadcast
        if biased:
            bq_s = const.tile([1, GC], BF16)
            bk_s = const.tile([1, GC], BF16)
            bv_s = const.tile([1, GC], BF16)
            ones_row = const.tile([1, QCH], BF16)
            ones_col = const.tile([1, P], BF16)

        # ---- input DMAs ----
        # One batched DMA per tensor/column-chunk, ordered so the first
        # projections (weights, then x columns for query-chunk 0, then the
        # context columns) unblock compute within a few us instead of after
        # the whole ~9 MB input load.
        nc.sync.dma_start(
            wqs.rearrange("p (kt g) -> p kt g", kt=KT),
            wq_d.rearrange("(kt p) g -> p kt g", p=P),
        )
        nc.sync.dma_start(
            wks.rearrange("p (kt g) -> p kt g", kt=KT),
            wk_d.rearrange("(kt p) g -> p kt g", p=P),
        )
        nc.sync.dma_start(
            wvs.rearrange("p (kt g) -> p kt g", kt=KT),
            wv_d.rearrange("(kt p) g -> p kt g", p=P),
        )
        nc.sync.dma_start(
            mks.rearrange("p (t q) -> p t q", t=4),
            msk_d.rearrange("(t p) q -> p t q", p=P),
        )
        xk_v = xk.rearrange("p (kt m) -> p kt m", kt=KT)
        xkvT_v = xkvT_d.rearrange("(kt p) m -> p kt m", p=P)
        for cc in (0, 4, 5, 1, 2, 3):
            nc.sync.dma_start(
                xk_v[:, :, cc * QCH:(cc + 1) * QCH],
                xkvT_v[:, :, cc * QCH:(cc + 1) * QCH],
            )
        nc.sync.dma_start(
            wos.rearrange("p (t d) -> p t d", t=2),
            wo_d.rearrange("(t p) d -> p t d", p=P),
        )
        nc.vector.memset(ones_bc[:], 1.0)
        if biased:
            nc.sync.dma_start(bq_s[:], bq_d[:])
            nc.sync.dma_start(bk_s[:], bk_d[:])
            nc.sync.dma_start(bv_s[:], bv_d[:])
            nc.vector.memset(ones_row[:], 1.0)
            nc.vector.memset(ones_col[:], 1.0)

        # ---- emission helpers ----
        def emit_qT_group(mt, c):
            psq = ps_main.tile([P, QCH], FP32, tag="proj", name="psq")
            for kt in range(KT):
                nc.tensor.matmul(
                    psq[:],
                    lhsT=wqs[:, kt * GC + mt * P: kt * GC + (mt + 1) * P],
                    rhs=xk[:, kt * NM + c * QCH: kt * NM + (c + 1) * QCH],
                    start=(kt == 0),
                    stop=(kt == KT - 1) and not biased,
                )
            if biased:
                nc.tensor.matmul(
                    psq[:], lhsT=bq_s[:, mt * P:(mt + 1) * P], rhs=ones_row[:],
                    start=False, stop=True,
                )
            nc.vector.tensor_copy(
                qT[:, mt * N + c * QCH: mt * N + (c + 1) * QCH], psq[:]
            )

        def emit_kT_group(mt, c2):
            psk = ps_main.tile([P, QCH], FP32, tag="proj", name="psk")
            for kt in range(KT):
                nc.tensor.matmul(
                    psk[:],
                    lhsT=wks[:, kt * GC + mt * P: kt * GC + (mt + 1) * P],
                    rhs=xk[:, kt * NM + c2 * QCH: kt * NM + (c2 + 1) * QCH],
                    start=(kt == 0),
                    stop=(kt == KT - 1) and not biased,
                )
            if biased:
                nc.tensor.matmul(
                    psk[:], lhsT=bk_s[:, mt * P:(mt + 1) * P], rhs=ones_row[:],
                    start=False, stop=True,
                )
            nc.vector.tensor_copy(
                kT[:, mt * NM + c2 * QCH: mt * NM + (c2 + 1) * QCH], psk[:]
            )

        def emit_v_group(t):
            psv = ps_main.tile([P, GC], FP32, tag="proj", name="psv")
            for kt in range(KT):
                nc.tensor.matmul(
                    psv[:],
                    lhsT=xk[:, kt * NM + t * P: kt * NM + (t + 1) * P],
                    rhs=wvs[:, kt * GC:(kt + 1) * GC],
                    start=(kt == 0),
                    stop=(kt == KT - 1) and not biased,
                )
            if biased:
                nc.tensor.matmul(
                    psv[:], lhsT=ones_col[:], rhs=bv_s[:], start=False, stop=True,
                )
            nc.vector.tensor_copy(vv[:, t * GC:(t + 1) * GC], psv[:])

        def emit_attention_chunk(c):
            kjs = _active_kj(c)
            last = len(kjs) - 1
            for pair in range(2):
                ps_acc = [None, None]
                den_acc = None
                pending = None  # (p_tile, i) exp'd tiles not yet fed to AV

                def do_av(pt, i):
                    # both heads concurrently via 128x64 column tiling:
                    # head hh lands on PSUM partitions hh*64..hh*64+63 of
                    # its own bank (separate banks -- the whole-bank
                    # has_written clear of start=True must not race the
                    # other head's accumulation).
                    t = kjs[i]
                    for hh in range(2):
                        h = pair * 2 + hh
                        lo = hh * 64
                        nc.tensor.matmul(
                            ps_acc[hh][lo:lo + 64, :],
                            lhsT=vv[:, t * GC + h * 64: t * GC + (h + 1) * 64],
                            rhs=pt[:, hh * QCH:(hh + 1) * QCH],
                            start=(i == 0),
                            stop=(i == last),
                        )

                for i, t in enumerate(kjs):
                    # both heads' scores into one 2-bank psum tile
                    pss = ps_s.tile([P, 2 * QCH], FP32, tag="s", name="pss")
                    for hh in range(2):
                        lo, hi = hh * 64, hh * 64 + 64
                        nc.tensor.matmul(
                            pss[:, hh * QCH:(hh + 1) * QCH],
                            lhsT=kT[lo:hi, pair * NM + t * P: pair * NM + (t + 1) * P],
                            rhs=qT[lo:hi, pair * N + c * QCH: pair * N + (c + 1) * QCH],
                            start=True,
                            stop=True,
                        )
                    pt = pexp.tile([P, 2 * QCH], BF16, tag="p", name="pt")
                    nc.scalar.activation(
                        pt[:], pss[:], mybir.ActivationFunctionType.Exp
                    )
                    if 4 * c <= t < 4 * c + 4:  # diagonal tile: causal mask
                        dt = t - 4 * c
                        for hh in range(2):
                            nc.vector.tensor_mul(
                                pt[:, hh * QCH:(hh + 1) * QCH],
                                pt[:, hh * QCH:(hh + 1) * QCH],
                                mks[:, dt * QCH:(dt + 1) * QCH],
                            )
                    # softmax denominator: elementwise accumulate the exp'd
                    # tiles (the cross-key reduction happens in one
                    # reduce+broadcast matmul per pair at chunk end).
                    if i == 0:
                        den_acc = bcp.tile(
                            [P, 2 * QCH], BF16, tag="dacc", bufs=2, name="dacc"
                        )
                        nc.vector.tensor_copy(den_acc[:], pt[:])
                    else:
                        nc.vector.tensor_add(den_acc[:], den_acc[:], pt[:])
                    if i == 0:
                        ps_acc[0] = ps_av.tile([P, QCH], FP32, tag="av", name="av0")
                        ps_acc[1] = ps_av.tile([P, QCH], FP32, tag="av", name="av1")
                    if pending is not None:
                        do_av(*pending)
                    pending = (pt, i)
                do_av(*pending)

                # normalize: one column-tiled matmul pair both reduces the
                # denominator over keys and broadcasts it to 64 partitions
                # per head; reciprocal_approx_fast runs on the broadcast
                # tile (free-dim cost is the same, and it skips the
                # ~3.3us/tile precise-reciprocal path entirely).
                dbc = ps_main.tile([P, QCH], FP32, tag="bc", name="dbc")
                for hh in range(2):
                    lo = hh * 64
                    nc.tensor.matmul(
                        dbc[lo:lo + 64, :],
                        lhsT=ones_bc[:, 0:64],
                        rhs=den_acc[:, hh * QCH:(hh + 1) * QCH],
                        start=True,
                        stop=True,
                    )
                dbs = bcp.tile([P, QCH], FP32, tag="dbs", name="dbs")
                nc.vector.tensor_copy(dbs[:], dbc[:])
                rbc = bcp.tile([P, QCH], FP32, tag="rbc", name="rbc")
                nc.vector.reciprocal_approx_fast(rbc[:], dbs[:])
                for hh in range(2):
                    lo = hh * 64
                    nc.vector.tensor_mul(
                        aT[lo:lo + 64, pair * N + c * QCH: pair * N + (c + 1) * QCH],
                        ps_acc[hh][lo:lo + 64, :],
                        rbc[lo:lo + 64, :],
                    )

        def emit_outproj_chunk(c):
            for it in range(4 * c, 4 * c + 4):
                for nh in range(2):
                    pso = ps_main.tile([P, QCH], FP32, tag="proj", name="pso")
                    for kt in range(2):
                        nc.tensor.matmul(
                            pso[:],
                            lhsT=aT[:, kt * N + it * P: kt * N + (it + 1) * P],
                            rhs=wos[:, kt * D + nh * QCH: kt * D + (nh + 1) * QCH],
                            start=(kt == 0),
                            stop=(kt == 1),
                        )
                    osb = pexp.tile([P, QCH], FP32, tag="osb", bufs=3, name="osb")
                    # split the psum evictions between ACT and DVE so
                    # neither queue serializes the out-projection tail
                    if nh == 0:
                        nc.scalar.copy(osb[:], pso[:])
                    else:
                        nc.vector.tensor_copy(osb[:], pso[:])
                    nc.sync.dma_start(
                        out_d[it * P:(it + 1) * P, nh * QCH:(nh + 1) * QCH], osb[:]
                    )

        # ---- interleaved emission: start attention as soon as its first
        # tiles exist; remaining projections become TensorE filler under the
        # ACT-bound attention rounds; out-projection trails each chunk ----
        for mt in range(2):
            emit_qT_group(mt, 0)
        for mt in range(2):
            emit_kT_group(mt, 0)
        for t in range(0, 4):
            emit_v_group(t)
        for c2 in (4, 5):
            for mt in range(2):
                emit_kT_group(mt, c2)
        for t in range(NSELF, NKJ):
            emit_v_group(t)
        for c in range(NQC):
            emit_attention_chunk(c)
            if c < NQC - 1:
                for mt in range(2):
                    emit_qT_group(mt, c + 1)
                for mt in range(2):
                    emit_kT_group(mt, c + 1)
                for t in range(4 * (c + 1), 4 * (c + 1) + 4):
                    emit_v_group(t)
            emit_outproj_chunk(c)

    nc.compile()
    return nc


_CACHE: dict = {}


def _module(biased: bool):
    if biased not in _CACHE:
        _CACHE[biased] = _build_module(biased)
    return _CACHE[biased]


def _mask_tiles():
    t = np.arange(4)[:, None, None]
    p = np.arange(P)[None, :, None]
    q = np.arange(QCH)[None, None, :]
    return (p + P * t <= q).astype(BF16NP).reshape(4 * P, QCH)


def kernel(x, context, Wq, bq, Wkv, bkv, Wo, bo, mask, context_mask):
    assert bool(np.all(mask)) and bool(np.all(context_mask)), (
        "only all-true padding masks are supported"
    )
    x = np.asarray(x, np.float32)
    context = np.asarray(context, np.float32)
    Wq, bq = np.asarray(Wq, np.float32), np.asarray(bq, np.float32)
    Wkv, bkv = np.asarray(Wkv, np.float32), np.asarray(bkv, np.float32)
    Wo, bo = np.asarray(Wo, np.float32), np.asarray(bo, np.float32)

    biased = bool(np.any(bq) or np.any(bkv))
    nc = _module(biased)

    msk = _mask_tiles()
    xkvT = [
        np.ascontiguousarray(
            np.concatenate([x[b], context[b]], axis=0).T.astype(BF16NP)
        )
        for b in range(B)
    ]
    in_maps = []
    for core in range(NCORES):
        b, g = divmod(core, GROUPS)
        cols = slice(g * GC, (g + 1) * GC)
        im = {
            "xkvT": xkvT[b],
            "wq": (Wq[:, cols] * SCALE).astype(BF16NP),
            "wk": Wkv[:, cols].astype(BF16NP),
            "wv": Wkv[:, D + g * GC: D + (g + 1) * GC].astype(BF16NP),
            "wo": np.ascontiguousarray(Wo[cols, :]).astype(BF16NP),
            "msk": msk,
        }
        if biased:
            im["bq"] = (bq[cols] * SCALE).astype(BF16NP).reshape(1, GC)
            im["bk"] = bkv[cols].astype(BF16NP).reshape(1, GC)
            im["bv"] = bkv[D + g * GC: D + (g + 1) * GC].astype(BF16NP).reshape(1, GC)
        in_maps.append(im)

    try:
        res = run_bass_kernel_spmd(nc, in_maps, core_ids=list(range(NCORES)))
    except ModuleNotFoundError:
        # BASS_TRACE set but the NTFF profiling hook isn't available in this
        # environment -- rerun with tracing hard-disabled.
        os.environ["BASS_NEVER_TRACE"] = "1"
        res = run_bass_kernel_spmd(nc, in_maps, core_ids=list(range(NCORES)))
    kernel.last_results = res
    out = np.zeros((B, N, D), np.float32)
    for core in range(NCORES):
        b = core // GROUPS
        out[b] += res.results[core]["out"]
    out += bo
    return out

